# revision 11
# baseline (speedup 1.0000x reference)
"""Trainium2 Bass kernel for nn_Net_75230647156948 (moe_routing).

Math (per batch row x of dim 64):
  xn   = (x - x_mean) / max(x_std, 1e-6)
  h1t  = tanh(xn @ bb_W1 + bb_b1)            [24]
  h    = tanh(h1t @ bb_W2 + bb_b2)           [16]
  g1t  = tanh(xn @ g_W1 + g_b1)              [12]
  l    = g1t @ g_W2 + g_b2                   [2]
  g0   = softmax(l)[0] = sigmoid(l0-l1) = (1+tanh((l0-l1)/2))/2
  o1   = tanh(h @ e1_W1 + e1_b1) @ e1_W2 + e1_b2     [3]
  o2   = tanh(h @ e2_W1 + e2_b1) @ e2_W2 + e2_b2     [3]
  y    = (g0*o1 + (1-g0)*o2 + 0.35*(xn @ sk_W + sk_b)) * y_std + y_mean

Rewritten for the device as y = S + td*F with
  td = tanh(dh),  dh = 0.5*(l0-l1)    (linear in g1t -> computed by matmul)
  F  = 0.5*(o1' - o2')                (o' = o scaled by y_std)
  S  = 0.5*(o1' + o2') + skip' + y_mean

Device dataflow (per core, pure data parallel over 8 cores):
  batch stays on the matmul moving (free) dim, features on partitions.
  p=2 batch blocks packed per 512-column tile (1024 rows/tile) using
  block-diagonal weights so K fills 128 partitions in stage 1.

kernel(**inputs) -> full [1048576, 3] float32 output.
Self-contained: hardcodes shapes; imports only installed packages.
"""

import sys

for _p in ("/opt/pypackages", "/opt/trn_rl_repo"):
    if _p not in sys.path:
        sys.path.insert(0, _p)

import numpy as np

import concourse.bass as bass  # noqa: F401  (bass must import before bacc)
import concourse.bacc as bacc
import concourse.mybir as mybir
import concourse.tile as tile

F32 = mybir.dt.float32
F32R = mybir.dt.float32r
BF16 = mybir.dt.bfloat16
TANH = mybir.ActivationFunctionType.Tanh

N_CORES = 8
BATCH = 1048576
D = 64
R_PER_CORE = BATCH // N_CORES  # 131072

# wp column layout
C_W1 = 0      # [128, 128]
C_W2 = 128    # rows 0-47, 32 cols
C_W3 = 160    # rows 64-127, 64 cols (50 real + 14 zero pad)
C_W4F = 224   # rows 0-69, 6 cols
C_W4T = 230   # rows 0-69, 6 cols
C_W4S = 236   # rows 0-69, 6 cols
WSPLIT = 242  # cols [0, WSPLIT) = matmul weights (sent as f32r image)
C_B1 = 242    # rows 0-127
C_B2 = 243    # rows 0-31
C_B3 = 244    # rows 0-63
C_ID = 245    # identity [128, 128]
NW = C_ID + 128  # 373
NWF = NW - WSPLIT  # f32 image cols (biases + identity)


def _prep_weights(inputs):
    """Fold norms/scales into the packed weight image wp [128, NW] f32."""
    f8 = np.float64
    g = {k: np.asarray(v, f8) for k, v in inputs.items() if k != "x"}
    s = 1.0 / np.maximum(g["x_std"], 1e-6)
    xms = g["x_mean"] * s

    def fold(W, b):
        return W * s[:, None], b - xms @ W

    bbW1, bbb1 = fold(g["bb_W1"], g["bb_b1"])
    gW1, gb1 = fold(g["g_W1"], g["g_b1"])
    skW, skb = fold(g["sk_W"], g["sk_b"])
    y_std, y_mean = g["y_std"], g["y_mean"]
    skWs = skW * (0.35 * y_std)[None, :]
    skc = 0.35 * y_std * skb + y_mean
    e1W2s = g["e1_W2"] * y_std[None, :]
    e1b2s = g["e1_b2"] * y_std
    e2W2s = g["e2_W2"] * y_std[None, :]
    e2b2s = g["e2_b2"] * y_std
    dvec = 0.5 * (g["g_W2"][:, 0] - g["g_W2"][:, 1])  # [12]
    dbias = 0.5 * (g["g_b2"][0] - g["g_b2"][1])

    wp = np.zeros((128, NW), f8)

    # ---- stage 1: lhsT [128, 128]; rhs = xT (A feats rows 0-63, B rows 64-127)
    # psum1/ws rows: 0-23 A.h1, 24-47 B.h1, 48-63 pad, 64-66 A.skip,
    # 67-69 B.skip, 70-95 pad (ws rows 64-95 overwritten by act2 with h),
    # 96-107 A.g1, 108-119 B.g1, 120-127 pad
    w1 = wp[:, C_W1:C_W1 + 128]
    w1[0:64, 0:24] = bbW1
    w1[64:128, 24:48] = bbW1
    w1[0:64, 64:67] = skWs
    w1[64:128, 67:70] = skWs
    w1[0:64, 96:108] = gW1
    w1[64:128, 108:120] = gW1

    # ---- stage 2: lhsT rows = ws[0:48]; cols 0-15 A.h, 16-31 B.h
    w2 = wp[:, C_W2:C_W2 + 32]
    w2[0:24, 0:16] = g["bb_W2"]
    w2[24:48, 16:32] = g["bb_W2"]

    # ---- stage 3: lhsT rows 64-127 = ws[64:128]
    # (ws: 64-79 A.h, 80-95 B.h, 96-107 A.g1t, 108-119 B.g1t, 120-127 zero)
    # psum3/s34 rows: 0-2 A.dh(x3), 3-5 B.dh(x3), 6-17 A.e1h, 18-29 A.e2h,
    # 30-41 B.e1h, 42-53 B.e2h, 54 ones-pre, 55-63 zero.
    # dh replicated 3x so act3 lands a broadcast td in SBUF for the final
    # DVE multiply (only one DVE operand may come from PSUM).
    w3 = wp[:, C_W3:C_W3 + 64]  # cols 54-63 stay zero
    for j in range(3):
        w3[96:108, j] = dvec
        w3[108:120, 3 + j] = dvec
    w3[64:80, 6:18] = g["e1_W1"]
    w3[64:80, 18:30] = g["e2_W1"]
    w3[80:96, 30:42] = g["e1_W1"]
    w3[80:96, 42:54] = g["e2_W1"]

    # ---- stage 4: rhs = s34[0:70]
    # s34 rows: 0-5 tdrep (A x3, B x3), 6-17 A.e1t, 18-29 A.e2t,
    # 30-41 B.e1t, 42-53 B.e2t, 54 ones (tanh(20)=1), 55-63 zeros,
    # 64-66 A.skip, 67-69 B.skip
    w4f = wp[:, C_W4F:C_W4F + 6]
    w4s = wp[:, C_W4S:C_W4S + 6]
    for j in range(3):
        # F = 0.5*(o1' - o2')
        w4f[6:18, j] = 0.5 * e1W2s[:, j]
        w4f[18:30, j] = -0.5 * e2W2s[:, j]
        w4f[54, j] = 0.5 * (e1b2s[j] - e2b2s[j])
        w4f[30:42, 3 + j] = 0.5 * e1W2s[:, j]
        w4f[42:54, 3 + j] = -0.5 * e2W2s[:, j]
        w4f[54, 3 + j] = 0.5 * (e1b2s[j] - e2b2s[j])
        # S = 0.5*(o1' + o2') + skip + const
        w4s[6:18, j] = 0.5 * e1W2s[:, j]
        w4s[18:30, j] = 0.5 * e2W2s[:, j]
        w4s[64 + j, j] = 1.0
        w4s[54, j] = 0.5 * (e1b2s[j] + e2b2s[j]) + skc[j]
        w4s[30:42, 3 + j] = 0.5 * e1W2s[:, j]
        w4s[42:54, 3 + j] = 0.5 * e2W2s[:, j]
        w4s[67 + j, 3 + j] = 1.0
        w4s[54, 3 + j] = 0.5 * (e1b2s[j] + e2b2s[j]) + skc[j]

    # ---- biases
    wp[0:24, C_B1] = bbb1
    wp[24:48, C_B1] = bbb1
    wp[96:108, C_B1] = gb1
    wp[108:120, C_B1] = gb1
    wp[0:16, C_B2] = g["bb_b2"]
    wp[16:32, C_B2] = g["bb_b2"]
    wp[0:6, C_B3] = dbias
    wp[6:18, C_B3] = g["e1_b1"]
    wp[18:30, C_B3] = g["e2_b1"]
    wp[30:42, C_B3] = g["e1_b1"]
    wp[42:54, C_B3] = g["e2_b1"]
    wp[54, C_B3] = 20.0  # tanh(20) == 1.0 in f32: free ones row via act3

    # ---- identity for PE transpose
    wp[:, C_ID:C_ID + 128] = np.eye(128)

    return np.ascontiguousarray(wp, np.float32)


def build_nc(rows, input_bf16=False):
    """Build the per-core Bass module for `rows` batch rows (mult of 1024).

    Matmuls run in float32r (TRN2 reduced-precision fp32, full rate at
    N>=256). All tensors feeding a matmul are allocated as f32r so their
    producers emit fp32r-rounded values (birverifier requirement).

    input_bf16: x arrives as bf16 in DRAM; load feature-major via DMA
    transpose (xbar), and stage 1 runs as a bf16 matmul against a bf16
    copy of W1 (extra input "w1b"). Otherwise x is f32, transposed on PE.
    """
    assert rows % 1024 == 0
    T = rows // 1024
    nc = bacc.Bacc("TRN2", target_bir_lowering=False, debug=False)
    x_dt = BF16 if input_bf16 else F32
    x_d = nc.dram_tensor("x", [rows, D], x_dt, kind="ExternalInput")
    wr_d = nc.dram_tensor("wpr", [128, WSPLIT], F32R, kind="ExternalInput")
    wf_d = nc.dram_tensor("wpf", [128, NWF], F32, kind="ExternalInput")
    if input_bf16:
        w1b_d = nc.dram_tensor("w1b", [128, 128], BF16, kind="ExternalInput")
    y_d = nc.dram_tensor("yt", [6, T * 512], F32, kind="ExternalOutput")

    with tile.TileContext(nc) as tc:
        with (
            tc.tile_pool(name="const", bufs=1) as const,
            tc.tile_pool(name="xin", bufs=8) as xin_pool,
            tc.tile_pool(name="xt", bufs=2) as xt_pool,
            tc.tile_pool(name="ws", bufs=2) as ws_pool,
            tc.tile_pool(name="s34", bufs=2) as s34_pool,
            tc.tile_pool(name="fin", bufs=4) as fin_pool,
            tc.tile_pool(name="pt", bufs=2, space="PSUM") as pt_pool,
            tc.tile_pool(name="p1", bufs=2, space="PSUM") as p1_pool,
            tc.tile_pool(name="p2", bufs=1, space="PSUM") as p2_pool,
            tc.tile_pool(name="p3", bufs=1, space="PSUM") as p3_pool,
            tc.tile_pool(name="p4", bufs=1, space="PSUM") as p4_pool,
        ):
            wpr = const.tile([128, WSPLIT], F32R)
            nc.sync.dma_start(wpr, wr_d[:, :])
            wpf = const.tile([128, NWF], F32)
            nc.sync.dma_start(wpf, wf_d[:, :])
            ident = wpf[:, C_ID - WSPLIT:C_ID - WSPLIT + 128]

            def bias_(c, lo, hi):
                return wpf[lo:hi, c - WSPLIT:c - WSPLIT + 1]

            if input_bf16:
                w1b = const.tile([128, 128], BF16)
                nc.sync.dma_start(w1b, w1b_d[:, :])

            for t in range(T):
                r0 = t * 1024
                # ---- load x feature-major: xT [128, 512],
                # rows 0-63 A feats, 64-127 B feats; col j = batch rows
                # (r0+j, r0+512+j)
                if input_bf16:
                    xT = xt_pool.tile([128, 512], BF16, tag="xt")
                    nc.sync.dma_start(xT[0:64, :], x_d[r0:r0 + 512, :],
                                      transpose=True)
                    nc.sync.dma_start(xT[64:128, :], x_d[r0 + 512:r0 + 1024, :],
                                      transpose=True)
                else:
                    pt = pt_pool.tile([128, 512], F32, tag="pt")
                    for c in range(4):
                        xi = xin_pool.tile([128, 128], F32, tag="xin")
                        ra = r0 + 128 * c
                        rb = r0 + 512 + 128 * c
                        nc.sync.dma_start(xi[:, 0:64], x_d[ra:ra + 128, :])
                        nc.sync.dma_start(xi[:, 64:128], x_d[rb:rb + 128, :])
                        nc.tensor.transpose(pt[:, 128 * c:128 * (c + 1)],
                                            xi, ident)
                    xT = xt_pool.tile([128, 512], F32R, tag="xt")
                    nc.scalar.copy(xT[:, 0:256], pt[:, 0:256])
                    nc.vector.tensor_copy(xT[:, 256:512], pt[:, 256:512])

                # ---- stage 1
                p1 = p1_pool.tile([128, 512], F32, tag="p1")
                if input_bf16:
                    nc.tensor.matmul(p1, w1b[:, :], xT[:, :])
                else:
                    nc.tensor.matmul(p1, wpr[:, C_W1:C_W1 + 128], xT)
                ws = ws_pool.tile([128, 512], F32R, tag="ws")
                nc.scalar.activation(ws, p1, TANH,
                                     bias=bias_(C_B1, 0, 128))

                # ---- stage 2. fp32r matmuls must write at psum partition
                # 0; act2 shifts the result up to ws[64:96] (engines support
                # partition-offset-shifting copies).
                p2 = p2_pool.tile([32, 512], F32, tag="p2")
                nc.tensor.matmul(p2, wpr[0:48, C_W2:C_W2 + 32], ws[0:48])
                nc.scalar.activation(ws[64:96], p2, TANH,
                                     bias=bias_(C_B2, 0, 32))

                # ---- stage 3
                p3 = p3_pool.tile([64, 512], F32, tag="p3")
                nc.tensor.matmul(p3, wpr[64:128, C_W3:C_W3 + 64],
                                 ws[64:128])
                s34 = s34_pool.tile([70, 512], F32R, tag="s34")
                nc.scalar.activation(s34[0:64], p3, TANH,
                                     bias=bias_(C_B3, 0, 64))
                nc.vector.tensor_copy(s34[64:70], p1[64:70])

                # ---- stage 4: F | S into one 2-bank psum tile
                p4 = p4_pool.tile([6, 1024], F32, tag="p4")
                nc.tensor.matmul(p4[:, 0:512], wpr[0:70, C_W4F:C_W4F + 6],
                                 s34)
                nc.tensor.matmul(p4[:, 512:1024], wpr[0:70, C_W4S:C_W4S + 6],
                                 s34)

                # ---- y = S + td*F   (td broadcast lives in s34[0:6], SBUF)
                prod = fin_pool.tile([6, 512], F32, tag="prod")
                nc.vector.tensor_mul(prod, p4[:, 0:512],
                                     s34[0:6].bitcast(F32))
                yb = fin_pool.tile([6, 512], F32, tag="yb")
                nc.vector.tensor_add(yb, prod, p4[:, 512:1024])
                nc.sync.dma_start(y_d[:, t * 512:(t + 1) * 512], yb)

    nc.compile()
    return nc


def unpack_out(yt, rows):
    """[6, T*512] device layout -> [rows, 3]."""
    T = rows // 1024
    a = np.asarray(yt, np.float32).reshape(2, 3, T, 512)
    out = np.empty((rows, 3), np.float32)
    v = out.reshape(T, 2, 512, 3)
    v[:, 0] = a[0].transpose(1, 2, 0)
    v[:, 1] = a[1].transpose(1, 2, 0)
    return out


class _Runner:
    """Cached PJRT executor for the SPMD kernel (mirrors
    bass2jax.run_bass_via_pjrt's multi-core path, but keeps the jitted
    executable and mesh so repeated calls don't re-trace)."""

    def __init__(self, rows, n_cores=N_CORES, input_bf16=False):
        import jax
        from jax.sharding import Mesh, PartitionSpec, NamedSharding
        from jax.experimental.shard_map import shard_map
        from concourse import bass2jax as b2j

        b2j.install_neuronx_cc_hook()
        self.input_bf16 = input_bf16
        nc = build_nc(rows, input_bf16=input_bf16)
        assert nc.dbg_addr is None
        part_name = (nc.partition_id_tensor.name
                     if nc.partition_id_tensor is not None else None)
        self.rows = rows
        self.n_cores = n_cores

        in_names, out_names, out_avals, zero_outs = [], [], [], []
        for alloc in nc.m.functions[0].allocations:
            if not isinstance(alloc, mybir.MemoryLocationSet):
                continue
            name = alloc.memorylocations[0].name
            if alloc.kind == "ExternalInput":
                if name != part_name:
                    in_names.append(name)
            elif alloc.kind == "ExternalOutput":
                shape = tuple(alloc.tensor_shape)
                dtype = mybir.dt.np(alloc.dtype)
                out_names.append(name)
                out_avals.append(jax.core.ShapedArray(shape, dtype))
                zero_outs.append(np.zeros(shape, dtype))
        n_params = len(in_names)
        all_names = in_names + out_names
        if part_name is not None:
            all_names = all_names + [part_name]

        def _body(*args):
            operands = list(args)
            if part_name is not None:
                operands.append(b2j.partition_id_tensor())
            outs = b2j._bass_exec_p.bind(
                *operands,
                out_avals=tuple(out_avals),
                in_names=tuple(all_names),
                out_names=tuple(out_names),
                lowering_input_output_aliases=(),
                sim_require_finite=True,
                sim_require_nnan=True,
                nc=nc,
            )
            return tuple(outs)

        devices = jax.devices()[:n_cores]
        assert len(devices) == n_cores
        mesh = Mesh(np.asarray(devices), ("core",))
        donate = tuple(range(n_params, n_params + len(out_names)))
        self._jit = jax.jit(
            shard_map(
                _body,
                mesh=mesh,
                in_specs=(PartitionSpec("core"),) * (n_params + len(out_names)),
                out_specs=(PartitionSpec("core"),) * len(out_names),
                check_rep=False,
            ),
            donate_argnums=donate,
            keep_unused=True,
        )
        self._jax = jax
        self._sharding = NamedSharding(mesh, PartitionSpec("core"))
        self.in_names = in_names
        self.out_names = out_names
        self.zero_outs = zero_outs

    def put_inputs(self, in_map_global):
        """Transfer global (n_cores*per_core) inputs to the devices."""
        return [
            self._jax.device_put(in_map_global[n], self._sharding)
            for n in self.in_names
        ]

    def make_zeros(self):
        return [
            self._jax.device_put(
                np.zeros((self.n_cores * z.shape[0], *z.shape[1:]), z.dtype),
                self._sharding,
            )
            for z in self.zero_outs
        ]

    def run_device(self, in_dev, zeros=None):
        """Execute once; returns dict of global outputs (jax arrays)."""
        if zeros is None:
            zeros = self.make_zeros()
        outs = self._jit(*in_dev, *zeros)
        return dict(zip(self.out_names, outs))


_RUNNER_CACHE = {}

# default execution variant; flipped after HW measurement if needed
INPUT_BF16 = False


def _get_runner(rows, input_bf16=None):
    if input_bf16 is None:
        input_bf16 = INPUT_BF16
    key = (rows, input_bf16)
    if key not in _RUNNER_CACHE:
        _RUNNER_CACHE[key] = _Runner(rows, input_bf16=input_bf16)
    return _RUNNER_CACHE[key]


def make_inputs_global(inputs, input_bf16=None):
    """Host-side prep: returns dict of global (8*per-core) input arrays."""
    if input_bf16 is None:
        input_bf16 = INPUT_BF16
    import ml_dtypes
    x = np.ascontiguousarray(np.asarray(inputs["x"], np.float32))
    assert x.shape == (BATCH, D)
    wp = _prep_weights(inputs)
    wpr = np.ascontiguousarray(wp[:, 0:WSPLIT])
    wpf = np.ascontiguousarray(wp[:, WSPLIT:NW])
    g = {"wpr": np.concatenate([wpr] * N_CORES, axis=0),
         "wpf": np.concatenate([wpf] * N_CORES, axis=0)}
    if input_bf16:
        g["x"] = x.astype(ml_dtypes.bfloat16)
        w1b = wp[:, C_W1:C_W1 + 128].astype(ml_dtypes.bfloat16)
        g["w1b"] = np.concatenate([w1b] * N_CORES, axis=0)
    else:
        g["x"] = x
    return g


def kernel(**inputs):
    runner = _get_runner(R_PER_CORE)
    in_dev = runner.put_inputs(make_inputs_global(inputs))
    outs = runner.run_device(in_dev)
    yt = np.asarray(outs["yt"])  # [8*6, T*512]
    T = R_PER_CORE // 1024
    return np.concatenate(
        [unpack_out(yt[6 * i:6 * (i + 1)], R_PER_CORE) for i in range(N_CORES)],
        axis=0,
    )
